# revision 1
# baseline (speedup 1.0000x reference)
"""MoE expert-collection kernel for 8 Trainium2 NeuronCores.

Problem (hardcoded shapes):
  x          [8192, 1024] f32
  expert_idx [8192]       int    (values 0..7)
  Wr         [8, 1024, 1024] f32, br [8, 1024] f32   (routing experts)
  Ws         [2, 1024, 1024] f32, bs [2, 1024] f32   (shared experts)
  out[n] = silu(x[n] @ Wr[e_n] + br[e_n]) + sum_s silu(x[n] @ Ws[s] + bs[s])

Strategy (expert parallel, host-side all-to-all):
  - Host sorts tokens by expert; core e computes silu(x @ Wr[e] + br[e]) for
    the tokens routed to expert e (padded to a common capacity C so all 8
    cores run one SPMD program).
  - The shared-expert work is data-parallel: core e also computes
    silu(x @ Ws[0] + bs[0]) + silu(x @ Ws[1] + bs[1]) for the fixed token
    slice [e*1024, (e+1)*1024) in original order (exactly 1/8 of the tokens,
    independent of routing, so the load is balanced).
  - Host combines: out = concat(shared slices); out[routed order] += routed.
  - Matmuls run in bf16 (fp32 PSUM accumulation): 1 PE cycle/row vs 4 for f32.
  - Layout: dout on PSUM partitions, tokens on the free dim, so the per-dout
    bias folds into the scalar-engine silu activation for free.
"""

import contextlib
import ctypes
import math
import sys
import types

import numpy as np
import ml_dtypes

import concourse.mybir as mybir
import concourse.tile as tile
from concourse import bacc
from concourse import bass_utils

N_CORES = 8
D = 1024          # d_in == d_out
P = 128           # partitions
KT = D // P       # 8 k-tiles
NJ = 3            # matrices per core: Wr[e], Ws[0], Ws[1]
N_EXPERTS = 8
S = 8192 // N_CORES  # shared-slice tokens per core (1024)

BF16 = mybir.dt.bfloat16
F32 = mybir.dt.float32

# exposed for test.py introspection
last_results = None
last_nc = None
last_in_maps = None

_program_cache = {}


def _install_ntff_hook_fallback():
    """Some containers (including this one) lack antenv.axon_hooks, but
    concourse's run_bass_kernel_spmd imports it unconditionally when tracing
    is requested (BASS_TRACE=1). Provide it: a ctypes port of
    trn_boot._ntff_profile_via_ctypes driving NRT profiling through the axon
    PJRT plugin, or a None hook (= trace gracefully skipped) if unavailable."""
    if "antenv.axon_hooks" in sys.modules:
        return
    try:
        import antenv.axon_hooks  # noqa: F401
        return
    except ImportError:
        pass

    hook = None
    try:
        lib = ctypes.CDLL("/opt/axon/libaxon_pjrt.so")
        if hasattr(lib, "axon_start_nrt_profile"):
            lib.axon_start_nrt_profile.argtypes = [
                ctypes.POINTER(ctypes.c_int64),
                ctypes.c_size_t,
            ]
            lib.axon_start_nrt_profile.restype = ctypes.c_int64
            lib.axon_stop_nrt_profile.argtypes = [ctypes.c_char_p]
            lib.axon_stop_nrt_profile.restype = ctypes.c_int64

            @contextlib.contextmanager
            def _hook(output_dir, device_ids):
                import jax

                jax.devices()  # force PJRT init so the axon client exists
                if device_ids:
                    ids = (ctypes.c_int64 * len(device_ids))(*device_ids)
                    rc = lib.axon_start_nrt_profile(ids, len(device_ids))
                else:
                    rc = lib.axon_start_nrt_profile(None, 0)
                if rc != 0:
                    raise RuntimeError(f"axon_start_nrt_profile rc={rc}")
                try:
                    yield
                finally:
                    n = lib.axon_stop_nrt_profile(str(output_dir).encode())
                    if n < 0:
                        raise RuntimeError(f"axon_stop_nrt_profile rc={n}")

            hook = _hook
    except OSError:
        pass

    mod = types.ModuleType("antenv.axon_hooks")
    mod.get_axon_ntff_profile_hook = lambda: hook
    mod.set_axon_ntff_profile_hook = lambda h: None
    sys.modules["antenv.axon_hooks"] = mod


_install_ntff_hook_fallback()


def _chunk_ranges(C, chunk=512):
    """Split [0, C) into PSUM-bank-sized chunks (<=512 each)."""
    out = []
    c0 = 0
    while c0 < C:
        c1 = min(c0 + chunk, C)
        out.append((c0, c1))
        c0 = c1
    return out


def _build_program(U):
    # xu holds the core's sorted-order shared window (cols 0:S) plus the
    # "extras" (routed tokens outside that window); both phases read it, so
    # x is loaded once. Routed tokens are processed in column blocks of
    # <=1536 so that any expert-count skew (large U) still fits SBUF and
    # the 8 PSUM banks.
    blocks = _chunk_ranges(U, 1536)
    xr_bufs = 2 if len(blocks) > 2 else 1
    s_chunks = _chunk_ranges(S)
    psum_bufs = 2

    nc = bacc.Bacc(
        "TRN2",
        target_bir_lowering=False,
        debug=False,
        enable_asserts=False,
        num_devices=N_CORES,
    )
    xu_d = nc.dram_tensor("xu", [D, U], BF16, kind="ExternalInput")
    W_d = nc.dram_tensor("W", [NJ, D, D], BF16, kind="ExternalInput")
    b_d = nc.dram_tensor("b", [P, NJ * KT], F32, kind="ExternalInput")
    outr_d = nc.dram_tensor("outr", [D, U], F32, kind="ExternalOutput")
    outs_d = nc.dram_tensor("outs", [D, S], F32, kind="ExternalOutput")

    with tile.TileContext(nc) as tc:
        with (
            tc.tile_pool(name="const", bufs=1) as constp,
            tc.tile_pool(name="wpool", bufs=1) as wp,
            tc.tile_pool(name="xpool", bufs=1) as xp,
            tc.tile_pool(name="silp", bufs=3) as silp,
            tc.tile_pool(name="outp", bufs=3) as outp,
            tc.tile_pool(name="psum", bufs=psum_bufs, space="PSUM") as psump,
        ):
            # --- PE warmup: ~8 dummy matmuls releasing the HAM clock throttle
            # while input DMAs stream in (PE would otherwise sit idle cold) ---
            warm_sb = constp.tile([P, 640], BF16, name="warm_sb")
            nc.vector.memset(warm_sb[:], 0.0)
            warm_ps = psump.tile([P, 512], F32, tag="warm", bufs=1, name="warm_ps")
            for i in range(10):
                nc.tensor.matmul(
                    warm_ps[:], warm_sb[:, :P], warm_sb[:, P : P + 512],
                    start=True, stop=True,
                )

            bias_t = constp.tile([P, NJ * KT], F32)

            # DMA order = first-use order: routed weights + block-0 xu first
            # (they gate the first matmuls), then the shared-phase weights.
            # Block 0 of xu doubles as the shared phase's input (cols 0:S),
            # so it gets its own persistent (bufs=1) slot.
            w_t = wp.tile([P, NJ, KT, D], BF16)
            b0_0, b1_0 = blocks[0]
            xr_t0 = xp.tile([P, KT, b1_0 - b0_0], BF16, tag="xrb0", name="xr_b0")
            for k in range(KT):
                nc.sync.dma_start(w_t[:, 0, k, :], W_d[0, k * P : (k + 1) * P, :])
                nc.sync.dma_start(xr_t0[:, k, :], xu_d[k * P : (k + 1) * P, b0_0:b1_0])
            xr_tiles = [xr_t0]
            for bi, (b0, b1) in enumerate(blocks[1:], start=1):
                xrb = xp.tile(
                    [P, KT, b1 - b0], BF16, tag="xrb", bufs=xr_bufs, name=f"xr_b{bi}"
                )
                for k in range(KT):
                    nc.sync.dma_start(
                        xrb[:, k, :], xu_d[k * P : (k + 1) * P, b0:b1]
                    )
                xr_tiles.append(xrb)
            # bias is tiny and first needed by the m=0 silu (~14us in), so it
            # loads after the ramp-critical W0/xu stream. W1/W2 are not
            # ramp-critical, so each loads as a single descriptor (saves the
            # per-DMA queue issue gaps of 8 separate transfers).
            nc.sync.dma_start(bias_t[:], b_d[:])
            nc.sync.dma_start(
                w_t[:, 1, :, :], W_d[1].rearrange("(kt p) n -> p kt n", p=P)
            )
            nc.sync.dma_start(
                w_t[:, 2, :, :], W_d[2].rearrange("(kt p) n -> p kt n", p=P)
            )

            # --- routed expert: j=0 over C sorted tokens (in column blocks) ---
            for bi, (b0, b1) in enumerate(blocks):
                xrb = xr_tiles[bi]
                r_chunks = _chunk_ranges(b1 - b0)
                for m in range(KT):
                    psums = [
                        psump.tile(
                            [P, c1 - c0], F32, tag=f"ps{ci}",
                            bufs=3 if ci == 0 else 2,
                            name=f"psr_{bi}_{m}_{ci}",
                        )
                        for ci, (c0, c1) in enumerate(r_chunks)
                    ]
                    for k in range(KT):
                        if bi == 0 and ((m == 0 and k > 0) or (m == 1 and k % 2 == 1)):
                            # dep-free filler matmul: occupies the PE during the
                            # DMA-limited ramp so the HAM clock stays unthrottled
                            nc.tensor.matmul(
                                warm_ps[:], warm_sb[:, :P], warm_sb[:, P : P + 512],
                                start=True, stop=True,
                            )
                        lhsT = w_t[:, 0, k, m * P : (m + 1) * P]
                        for ci, (c0, c1) in enumerate(r_chunks):
                            nc.tensor.matmul(
                                psums[ci][:],
                                lhsT,
                                xrb[:, k, c0:c1],
                                start=(k == 0),
                                stop=(k == KT - 1),
                            )
                    outr_t = outp.tile(
                        [P, b1 - b0], F32, tag="outr", name=f"outr_{bi}_{m}"
                    )
                    for ci, (c0, c1) in enumerate(r_chunks):
                        nc.scalar.activation(
                            outr_t[:, c0:c1],
                            psums[ci][:],
                            mybir.ActivationFunctionType.Silu,
                            bias=bias_t[:, m : m + 1],
                        )
                    # stores go on the scalar HWDGE queue: the sync queue is
                    # saturated with input loads early on and FIFO head-of-line
                    # blocking there stalls the silu -> psum-recycle chain
                    # (and SWDGE stores cost a ~5us gpsimd drain at kernel end).
                    # Chunked so the store of chunk i overlaps the silu of i+1.
                    for ci, (c0, c1) in enumerate(r_chunks):
                        nc.scalar.dma_start(
                            outr_d[m * P : (m + 1) * P, b0 + c0 : b0 + c1],
                            outr_t[:, c0:c1],
                        )

            # --- shared experts: j=1,2 over the fixed S-token slice ---
            for m in range(KT):
                sils = []
                for j in (1, 2):
                    psums = [
                        psump.tile(
                            [P, c1 - c0], F32, tag=f"ps{ci}",
                            bufs=3 if ci == 0 else 2,
                            name=f"pss_{m}_{j}_{ci}",
                        )
                        for ci, (c0, c1) in enumerate(s_chunks)
                    ]
                    for k in range(KT):
                        lhsT = w_t[:, j, k, m * P : (m + 1) * P]
                        for ci, (c0, c1) in enumerate(s_chunks):
                            nc.tensor.matmul(
                                psums[ci][:],
                                lhsT,
                                xr_tiles[0][:, k, c0:c1],
                                start=(k == 0),
                                stop=(k == KT - 1),
                            )
                    sil = silp.tile([P, S], F32, tag=f"sil{j}", name=f"sil_{m}_{j}")
                    bidx = j * KT + m
                    for ci, (c0, c1) in enumerate(s_chunks):
                        nc.scalar.activation(
                            sil[:, c0:c1],
                            psums[ci][:],
                            mybir.ActivationFunctionType.Silu,
                            bias=bias_t[:, bidx : bidx + 1],
                        )
                    sils.append(sil)
                outs_t = outp.tile([P, S], F32, tag="outs", name=f"outs_{m}")
                for ci, (c0, c1) in enumerate(s_chunks):
                    nc.vector.tensor_add(
                        outs_t[:, c0:c1], sils[0][:, c0:c1], sils[1][:, c0:c1]
                    )
                    # the sync queue is idle once inputs finish (~34us), so
                    # shared-phase stores avoid the scalar queue's silu FIFO
                    nc.sync.dma_start(
                        outs_d[m * P : (m + 1) * P, c0:c1], outs_t[:, c0:c1]
                    )

    nc.compile()
    return nc


def _get_program(C):
    if C not in _program_cache:
        _program_cache[C] = _build_program(C)
    return _program_cache[C]


def kernel(x, expert_idx, Wr, br, Ws, bs):
    global last_results, last_nc, last_in_maps

    x = np.asarray(x, dtype=np.float32)
    idx = np.asarray(expert_idx).astype(np.int64)
    Wr = np.asarray(Wr, dtype=np.float32)
    br = np.asarray(br, dtype=np.float32)
    Ws = np.asarray(Ws, dtype=np.float32)
    bs = np.asarray(bs, dtype=np.float32)

    n_tokens = x.shape[0]
    assert x.shape == (N_CORES * S, D), f"unexpected x shape {x.shape}"

    # --- host-side "all-to-all": group tokens by expert ---
    order = np.argsort(idx, kind="stable")
    counts = np.bincount(idx, minlength=N_EXPERTS)
    offsets = np.zeros(N_EXPERTS + 1, dtype=np.int64)
    np.cumsum(counts, out=offsets[1:])

    x_sorted_bf = x[order].astype(ml_dtypes.bfloat16)

    # Core e's xu = [sorted window e*S:(e+1)*S | extras], where extras are
    # the routed (expert-e) tokens falling outside that window. The routed
    # phase computes all U columns with Wr[e]; only the expert-e ones are
    # kept, so the window+extras layout lets one x load serve both phases.
    extras = []
    cols = []  # per core: xu column of each routed token (sorted order)
    for e in range(N_CORES):
        p = np.arange(offsets[e], offsets[e + 1])
        inside = (p >= e * S) & (p < (e + 1) * S)
        ex = p[~inside]
        col = np.where(inside, p - e * S, 0)
        col[~inside] = S + np.arange(len(ex))
        extras.append(ex)
        cols.append(col)
    E = max(32, int(math.ceil(max(len(ex) for ex in extras) / 32)) * 32)
    U = S + E

    Wr_bf = Wr.astype(ml_dtypes.bfloat16)
    Ws_bf = Ws.astype(ml_dtypes.bfloat16)

    in_maps = []
    for e in range(N_CORES):
        xu = np.zeros((D, U), dtype=ml_dtypes.bfloat16)
        xu[:, :S] = x_sorted_bf[e * S : (e + 1) * S].T
        if len(extras[e]):
            xu[:, S : S + len(extras[e])] = x_sorted_bf[extras[e]].T

        W = np.empty((NJ, D, D), dtype=ml_dtypes.bfloat16)
        W[0] = Wr_bf[e]
        W[1] = Ws_bf[0]
        W[2] = Ws_bf[1]

        # b[p, j*KT + m] = bias_j[m*P + p]
        b = np.empty((P, NJ * KT), dtype=np.float32)
        for j, bias in enumerate((br[e], bs[0], bs[1])):
            b[:, j * KT : (j + 1) * KT] = bias.reshape(KT, P).T

        in_maps.append({"xu": xu, "W": W, "b": b})

    nc = _get_program(U)
    res = bass_utils.run_bass_kernel_spmd(nc, in_maps, core_ids=list(range(N_CORES)))
    last_results = res
    last_nc = nc
    last_in_maps = in_maps

    # combine in sorted-token space, then permute back to input order
    out_sorted = np.concatenate(
        [res.results[e]["outs"].T for e in range(N_CORES)], axis=0
    ).astype(np.float32, copy=False)
    for e in range(N_CORES):
        if counts[e] == 0:
            continue
        out_sorted[offsets[e] : offsets[e + 1]] += res.results[e]["outr"][
            :, cols[e]
        ].T
    out = np.empty_like(out_sorted)
    out[order] = out_sorted
    return out[:n_tokens]



# revision 2
# speedup vs baseline: 1.0036x; 1.0036x over previous
"""MoE expert-collection kernel for 8 Trainium2 NeuronCores.

Problem (hardcoded shapes):
  x          [8192, 1024] f32
  expert_idx [8192]       int    (values 0..7)
  Wr         [8, 1024, 1024] f32, br [8, 1024] f32   (routing experts)
  Ws         [2, 1024, 1024] f32, bs [2, 1024] f32   (shared experts)
  out[n] = silu(x[n] @ Wr[e_n] + br[e_n]) + sum_s silu(x[n] @ Ws[s] + bs[s])

Strategy (expert parallel, host-side all-to-all):
  - Host sorts tokens by expert; core e computes silu(x @ Wr[e] + br[e]) for
    the tokens routed to expert e (padded to a common capacity U so all 8
    cores run one SPMD program).
  - The shared-expert work is data-parallel: core e also computes
    silu(x @ Ws[0] + bs[0]) + silu(x @ Ws[1] + bs[1]) for the fixed token
    slice [e*1024, (e+1)*1024) in sorted order (cols 0:S of xu), so the load
    is balanced and x is loaded once (window + routed extras).
  - Host combines: out = concat(shared slices); out[routed order] += routed.
  - Matmuls run in bf16 (fp32 PSUM accumulation): 1 PE cycle/row vs 4 for f32.

Schedule (from trace analysis of the previous version):
  - Phase order: shared j=0 (Ws0) -> shared j=1 (Ws1) -> routed (Wr[e]).
    The routed phase ends on the tiny extras chunk, so the end-of-kernel
    silu->store drain is ~2us instead of ~7us.
  - The opening phases are k-OUTER over 512-token column chunks with all 8
    PSUM banks live (two sets of 4 m-tiles), so the first matmul only needs
    one W k-tile + one xu k-tile chunk instead of the whole 4MB working set.
    The PE starts on real data right after the engine preamble and the cold
    HAM ramp doubles as warmup (no dummy matmuls needed).
  - Weight loads go on the scalar HWDGE ring, activation loads + all stores
    on the sync ring: the two issue streams run in parallel during the ramp,
    and stores only begin after all loads have drained.
  - Outputs are stored as bf16 (host upcasts): halves store traffic so DMA
    never contends with the weight prefetch (the previous version stalled
    1.5us waiting for Ws under fp32-store traffic).
"""

import contextlib
import ctypes
import math
import sys
import types

import numpy as np
import ml_dtypes

import concourse.mybir as mybir
import concourse.tile as tile
from concourse import bacc
from concourse import bass_utils

N_CORES = 8
D = 1024          # d_in == d_out
P = 128           # partitions
KT = D // P       # 8 k-tiles
NJ = 3            # matrices per core: Ws[0], Ws[1], Wr[e]
N_EXPERTS = 8
S = 8192 // N_CORES  # shared-slice tokens per core (1024)
CH = 512          # PSUM bank chunk (512 fp32)

BF16 = mybir.dt.bfloat16
F32 = mybir.dt.float32

# exposed for test.py introspection
last_results = None
last_nc = None
last_in_maps = None

_program_cache = {}


def _install_ntff_hook_fallback():
    """Some containers (including this one) lack antenv.axon_hooks, but
    concourse's run_bass_kernel_spmd imports it unconditionally when tracing
    is requested (BASS_TRACE=1). Provide it: a ctypes port of
    trn_boot._ntff_profile_via_ctypes driving NRT profiling through the axon
    PJRT plugin, or a None hook (= trace gracefully skipped) if unavailable."""
    if "antenv.axon_hooks" in sys.modules:
        return
    try:
        import antenv.axon_hooks  # noqa: F401
        return
    except ImportError:
        pass

    hook = None
    try:
        lib = ctypes.CDLL("/opt/axon/libaxon_pjrt.so")
        if hasattr(lib, "axon_start_nrt_profile"):
            lib.axon_start_nrt_profile.argtypes = [
                ctypes.POINTER(ctypes.c_int64),
                ctypes.c_size_t,
            ]
            lib.axon_start_nrt_profile.restype = ctypes.c_int64
            lib.axon_stop_nrt_profile.argtypes = [ctypes.c_char_p]
            lib.axon_stop_nrt_profile.restype = ctypes.c_int64

            @contextlib.contextmanager
            def _hook(output_dir, device_ids):
                import jax

                jax.devices()  # force PJRT init so the axon client exists
                if device_ids:
                    ids = (ctypes.c_int64 * len(device_ids))(*device_ids)
                    rc = lib.axon_start_nrt_profile(ids, len(device_ids))
                else:
                    rc = lib.axon_start_nrt_profile(None, 0)
                if rc != 0:
                    raise RuntimeError(f"axon_start_nrt_profile rc={rc}")
                try:
                    yield
                finally:
                    n = lib.axon_stop_nrt_profile(str(output_dir).encode())
                    if n < 0:
                        raise RuntimeError(f"axon_stop_nrt_profile rc={n}")

            hook = _hook
    except OSError:
        pass

    mod = types.ModuleType("antenv.axon_hooks")
    mod.get_axon_ntff_profile_hook = lambda: hook
    mod.set_axon_ntff_profile_hook = lambda h: None
    sys.modules["antenv.axon_hooks"] = mod


_install_ntff_hook_fallback()


def _chunk_ranges(c0, c1, chunk=CH):
    out = []
    while c0 < c1:
        out.append((c0, min(c0 + chunk, c1)))
        c0 = out[-1][1]
    return out


def _build_program(U):
    nc = bacc.Bacc(
        "TRN2",
        target_bir_lowering=False,
        debug=False,
        enable_asserts=False,
        num_devices=N_CORES,
    )
    xu_d = nc.dram_tensor("xu", [D, U], BF16, kind="ExternalInput")
    W_d = nc.dram_tensor("W", [NJ, D, D], BF16, kind="ExternalInput")
    b_d = nc.dram_tensor("b", [P, NJ * KT], F32, kind="ExternalInput")
    outr_d = nc.dram_tensor("outr", [D, U], BF16, kind="ExternalOutput")
    outs_d = nc.dram_tensor("outs", [D, S], BF16, kind="ExternalOutput")

    with tile.TileContext(nc) as tc:
        with (
            tc.tile_pool(name="const", bufs=1) as constp,
            tc.tile_pool(name="wpool", bufs=1) as wp,
            tc.tile_pool(name="xpool", bufs=1) as xp,
            tc.tile_pool(name="sil1p", bufs=1) as sil1p,
            tc.tile_pool(name="sil2p", bufs=3) as sil2p,
            tc.tile_pool(name="outp", bufs=4) as outp,
            tc.tile_pool(name="psum", bufs=1, space="PSUM") as psump,
        ):
            w_t = wp.tile([P, NJ, KT, D], BF16)
            xu_t = xp.tile([P, KT, U], BF16)
            bias_t = constp.tile([P, NJ * KT], F32)
            sil1_t = sil1p.tile([P, KT, S], BF16)

            # --- input DMA: two parallel HWDGE issue streams ---
            # scalar ring: weights (first MM gates on W0k0; bias before the
            # bulk so the first silu never waits on it)
            nc.scalar.dma_start(w_t[:, 0, 0, :], W_d[0, 0:P, :])
            nc.scalar.dma_start(bias_t[:], b_d[:])
            for k in range(1, KT):
                nc.scalar.dma_start(w_t[:, 0, k, :], W_d[0, k * P : (k + 1) * P, :])
            nc.scalar.dma_start(
                w_t[:, 1, :, :], W_d[1].rearrange("(kt p) n -> p kt n", p=P)
            )
            nc.scalar.dma_start(
                w_t[:, 2, :, :], W_d[2].rearrange("(kt p) n -> p kt n", p=P)
            )
            # sync ring: activations (per-k chunk 0 paces the opening phase,
            # the rest lands long before it is needed)
            for k in range(KT):
                nc.sync.dma_start(
                    xu_t[:, k, 0:CH], xu_d[k * P : (k + 1) * P, 0:CH]
                )
            nc.sync.dma_start(
                xu_t[:, :, CH:S],
                xu_d[:, CH:S].rearrange("(kt p) n -> p kt n", p=P),
            )
            nc.sync.dma_start(
                xu_t[:, :, S:U],
                xu_d[:, S:U].rearrange("(kt p) n -> p kt n", p=P),
            )

            # --- shared experts, k-outer: phase = (j, col-chunk, m-group) ---
            # 8 PSUM banks = 2 sets x 4 m-tiles; sets alternate per phase so a
            # bank's silu has a full phase (~7us) to drain before reuse.
            for j in range(2):
                for c0, c1 in ((0, CH), (CH, S)):
                    for mg in range(2):
                        ps = [
                            psump.tile(
                                [P, CH], F32, tag=f"ps{mg * 4 + i}",
                                name=f"pss_{j}_{c0}_{mg}_{i}",
                            )
                            for i in range(4)
                        ]
                        for k in range(KT):
                            for i in range(4):
                                m = mg * 4 + i
                                nc.tensor.matmul(
                                    ps[i][:],
                                    w_t[:, j, k, m * P : (m + 1) * P],
                                    xu_t[:, k, c0:c1],
                                    start=(k == 0),
                                    stop=(k == KT - 1),
                                )
                        for i in range(4):
                            m = mg * 4 + i
                            bidx = j * KT + m
                            if j == 0:
                                nc.scalar.activation(
                                    sil1_t[:, m, c0:c1],
                                    ps[i][:],
                                    mybir.ActivationFunctionType.Silu,
                                    bias=bias_t[:, bidx : bidx + 1],
                                )
                            else:
                                sil2 = sil2p.tile(
                                    [P, CH], BF16, tag="sil2",
                                    name=f"sil2_{c0}_{m}",
                                )
                                nc.scalar.activation(
                                    sil2[:],
                                    ps[i][:],
                                    mybir.ActivationFunctionType.Silu,
                                    bias=bias_t[:, bidx : bidx + 1],
                                )
                                outs_t = outp.tile(
                                    [P, CH], BF16, tag="outs",
                                    name=f"outs_{c0}_{m}",
                                )
                                nc.vector.tensor_add(
                                    outs_t[:], sil1_t[:, m, c0:c1], sil2[:]
                                )
                                nc.sync.dma_start(
                                    outs_d[m * P : (m + 1) * P, c0:c1], outs_t[:]
                                )

            # --- routed expert, m-outer k-inner (mid-kernel, DMA caught up):
            # per (m,k) the 3 column chunks share one weight load; the final
            # chunk is the small extras block, so the tail drain is tiny.
            r_chunks = _chunk_ranges(0, U)
            tag_ctr = 0
            for m in range(KT):
                psums = []
                for ci, (c0, c1) in enumerate(r_chunks):
                    psums.append(
                        psump.tile(
                            [P, c1 - c0], F32, tag=f"ps{tag_ctr % 8}",
                            name=f"psr_{m}_{ci}",
                        )
                    )
                    tag_ctr += 1
                for k in range(KT):
                    lhsT = w_t[:, 2, k, m * P : (m + 1) * P]
                    for ci, (c0, c1) in enumerate(r_chunks):
                        nc.tensor.matmul(
                            psums[ci][:],
                            lhsT,
                            xu_t[:, k, c0:c1],
                            start=(k == 0),
                            stop=(k == KT - 1),
                        )
                bidx = 2 * KT + m
                for ci, (c0, c1) in enumerate(r_chunks):
                    outr_t = outp.tile(
                        [P, c1 - c0], BF16, tag="outr", name=f"outr_{m}_{ci}"
                    )
                    nc.scalar.activation(
                        outr_t[:],
                        psums[ci][:],
                        mybir.ActivationFunctionType.Silu,
                        bias=bias_t[:, bidx : bidx + 1],
                    )
                    nc.sync.dma_start(
                        outr_d[m * P : (m + 1) * P, c0:c1], outr_t[:]
                    )

    nc.compile()
    return nc


def _get_program(U):
    if U not in _program_cache:
        _program_cache[U] = _build_program(U)
    return _program_cache[U]


def kernel(x, expert_idx, Wr, br, Ws, bs):
    global last_results, last_nc, last_in_maps

    x = np.asarray(x, dtype=np.float32)
    idx = np.asarray(expert_idx).astype(np.int64)
    Wr = np.asarray(Wr, dtype=np.float32)
    br = np.asarray(br, dtype=np.float32)
    Ws = np.asarray(Ws, dtype=np.float32)
    bs = np.asarray(bs, dtype=np.float32)

    n_tokens = x.shape[0]
    assert x.shape == (N_CORES * S, D), f"unexpected x shape {x.shape}"

    # --- host-side "all-to-all": group tokens by expert ---
    order = np.argsort(idx, kind="stable")
    counts = np.bincount(idx, minlength=N_EXPERTS)
    offsets = np.zeros(N_EXPERTS + 1, dtype=np.int64)
    np.cumsum(counts, out=offsets[1:])

    x_sorted_bf = x[order].astype(ml_dtypes.bfloat16)

    # Core e's xu = [sorted window e*S:(e+1)*S | extras], where extras are
    # the routed (expert-e) tokens falling outside that window. The routed
    # phase computes all U columns with Wr[e]; only the expert-e ones are
    # kept, so the window+extras layout lets one x load serve both phases.
    extras = []
    cols = []  # per core: xu column of each routed token (sorted order)
    for e in range(N_CORES):
        p = np.arange(offsets[e], offsets[e + 1])
        inside = (p >= e * S) & (p < (e + 1) * S)
        ex = p[~inside]
        col = np.where(inside, p - e * S, 0)
        col[~inside] = S + np.arange(len(ex))
        extras.append(ex)
        cols.append(col)
    E = max(32, int(math.ceil(max(len(ex) for ex in extras) / 32)) * 32)
    U = S + E

    Wr_bf = Wr.astype(ml_dtypes.bfloat16)
    Ws_bf = Ws.astype(ml_dtypes.bfloat16)

    in_maps = []
    for e in range(N_CORES):
        xu = np.zeros((D, U), dtype=ml_dtypes.bfloat16)
        xu[:, :S] = x_sorted_bf[e * S : (e + 1) * S].T
        if len(extras[e]):
            xu[:, S : S + len(extras[e])] = x_sorted_bf[extras[e]].T

        W = np.empty((NJ, D, D), dtype=ml_dtypes.bfloat16)
        W[0] = Ws_bf[0]
        W[1] = Ws_bf[1]
        W[2] = Wr_bf[e]

        # b[p, j*KT + m] = bias_j[m*P + p]
        b = np.empty((P, NJ * KT), dtype=np.float32)
        for j, bias in enumerate((bs[0], bs[1], br[e])):
            b[:, j * KT : (j + 1) * KT] = bias.reshape(KT, P).T

        in_maps.append({"xu": xu, "W": W, "b": b})

    nc = _get_program(U)
    res = bass_utils.run_bass_kernel_spmd(nc, in_maps, core_ids=list(range(N_CORES)))
    last_results = res
    last_nc = nc
    last_in_maps = in_maps

    # combine in sorted-token space, then permute back to input order
    out_sorted = np.concatenate(
        [np.asarray(res.results[e]["outs"]).astype(np.float32).T for e in range(N_CORES)],
        axis=0,
    )
    for e in range(N_CORES):
        if counts[e] == 0:
            continue
        out_sorted[offsets[e] : offsets[e + 1]] += (
            np.asarray(res.results[e]["outr"]).astype(np.float32)[:, cols[e]].T
        )
    out = np.empty_like(out_sorted)
    out[order] = out_sorted
    return out[:n_tokens]


# revision 6
# speedup vs baseline: 1.0273x; 1.0236x over previous
"""MoE expert-collection kernel for 8 Trainium2 NeuronCores.

Problem (hardcoded shapes):
  x          [8192, 1024] f32
  expert_idx [8192]       int    (values 0..7)
  Wr         [8, 1024, 1024] f32, br [8, 1024] f32   (routing experts)
  Ws         [2, 1024, 1024] f32, bs [2, 1024] f32   (shared experts)
  out[n] = silu(x[n] @ Wr[e_n] + br[e_n]) + sum_s silu(x[n] @ Ws[s] + bs[s])

Strategy (expert parallel, host-side all-to-all):
  - Host sorts tokens by expert; core e computes silu(x @ Wr[e] + br[e]) for
    the tokens routed to expert e (padded to a common capacity U so all 8
    cores run one SPMD program).
  - The shared-expert work is data-parallel: core e also computes
    silu(x @ Ws[0] + bs[0]) + silu(x @ Ws[1] + bs[1]) for the fixed token
    slice [e*1024, (e+1)*1024) in sorted order (cols 0:S of xu), so the load
    is balanced and x is loaded once (window + routed extras).
  - Host combines: out = concat(shared slices); out[routed order] += routed.
  - Matmuls run in bf16 (fp32 PSUM accumulation): 1 PE cycle/row vs 4 for f32.

Schedule (from trace analysis of the previous version):
  - Phase order: shared j=0 (Ws0) -> shared j=1 (Ws1) -> routed (Wr[e]).
    The routed phase ends on the tiny extras chunk, so the end-of-kernel
    silu->store drain is ~2us instead of ~7us.
  - The opening phases are k-OUTER over 512-token column chunks with all 8
    PSUM banks live (two sets of 4 m-tiles), so the first matmul only needs
    half a W k-tile + one xu k-tile chunk instead of the whole 4MB working
    set. The PE starts on real data right after the engine preamble and the
    cold HAM ramp doubles as warmup (no dummy matmuls needed).
  - Weight loads go on the scalar HWDGE ring, activation loads + all stores
    on the sync ring: the two issue streams run in parallel during the ramp,
    and stores only begin after all loads have drained. W k-tiles are split
    into m-group halves so the ramp phase's data demand is paced to what it
    actually consumes (a full-k W DMA stream starved the PE and let HAM
    re-throttle mid-ramp).
  - xu is staged host-side in [P, KT, U] layout so every DMA is contiguous
    >=1KB runs per partition (the [D, U]-layout extras gather degenerated to
    64B strided reads that hogged the SDMA engines during the ramp).
  - Outputs are stored as bf16 (host upcasts): halves store traffic so DMA
    never contends with the weight prefetch (the fp32-store version stalled
    1.5us waiting for Ws under store traffic).
"""

import contextlib
import ctypes
import math
import sys
import types

import numpy as np
import ml_dtypes

import concourse.mybir as mybir
import concourse.tile as tile
from concourse import bacc
from concourse import bass_utils

N_CORES = 8
D = 1024          # d_in == d_out
P = 128           # partitions
KT = D // P       # 8 k-tiles
NJ = 3            # matrices per core: Ws[0], Ws[1], Wr[e]
N_EXPERTS = 8
S = 8192 // N_CORES  # shared-slice tokens per core (1024)
CH = 512          # PSUM bank chunk (512 fp32)

BF16 = mybir.dt.bfloat16
F32 = mybir.dt.float32

# exposed for test.py introspection
last_results = None
last_nc = None
last_in_maps = None

_program_cache = {}


def _install_ntff_hook_fallback():
    """Some containers (including this one) lack antenv.axon_hooks, but
    concourse's run_bass_kernel_spmd imports it unconditionally when tracing
    is requested (BASS_TRACE=1). Provide it: a ctypes port of
    trn_boot._ntff_profile_via_ctypes driving NRT profiling through the axon
    PJRT plugin, or a None hook (= trace gracefully skipped) if unavailable."""
    if "antenv.axon_hooks" in sys.modules:
        return
    try:
        import antenv.axon_hooks  # noqa: F401
        return
    except ImportError:
        pass

    hook = None
    try:
        lib = ctypes.CDLL("/opt/axon/libaxon_pjrt.so")
        if hasattr(lib, "axon_start_nrt_profile"):
            lib.axon_start_nrt_profile.argtypes = [
                ctypes.POINTER(ctypes.c_int64),
                ctypes.c_size_t,
            ]
            lib.axon_start_nrt_profile.restype = ctypes.c_int64
            lib.axon_stop_nrt_profile.argtypes = [ctypes.c_char_p]
            lib.axon_stop_nrt_profile.restype = ctypes.c_int64

            @contextlib.contextmanager
            def _hook(output_dir, device_ids):
                import jax

                jax.devices()  # force PJRT init so the axon client exists
                if device_ids:
                    ids = (ctypes.c_int64 * len(device_ids))(*device_ids)
                    rc = lib.axon_start_nrt_profile(ids, len(device_ids))
                else:
                    rc = lib.axon_start_nrt_profile(None, 0)
                if rc != 0:
                    raise RuntimeError(f"axon_start_nrt_profile rc={rc}")
                try:
                    yield
                finally:
                    n = lib.axon_stop_nrt_profile(str(output_dir).encode())
                    if n < 0:
                        raise RuntimeError(f"axon_stop_nrt_profile rc={n}")

            hook = _hook
    except OSError:
        pass

    mod = types.ModuleType("antenv.axon_hooks")
    mod.get_axon_ntff_profile_hook = lambda: hook
    mod.set_axon_ntff_profile_hook = lambda h: None
    sys.modules["antenv.axon_hooks"] = mod


_install_ntff_hook_fallback()


def _chunk_ranges(c0, c1, chunk=CH):
    out = []
    while c0 < c1:
        out.append((c0, min(c0 + chunk, c1)))
        c0 = out[-1][1]
    return out


def _build_program(U):
    nc = bacc.Bacc(
        "TRN2",
        target_bir_lowering=False,
        debug=False,
        enable_asserts=False,
        num_devices=N_CORES,
    )
    xu_d = nc.dram_tensor("xu", [P, KT, U], BF16, kind="ExternalInput")
    W_d = nc.dram_tensor("W", [NJ, D, D], BF16, kind="ExternalInput")
    b_d = nc.dram_tensor("b", [P, NJ * KT], F32, kind="ExternalInput")
    outr_d = nc.dram_tensor("outr", [D, U], BF16, kind="ExternalOutput")
    outs_d = nc.dram_tensor("outs", [D, S], BF16, kind="ExternalOutput")

    with tile.TileContext(nc) as tc:
        with (
            tc.tile_pool(name="const", bufs=1) as constp,
            tc.tile_pool(name="wpool", bufs=1) as wp,
            tc.tile_pool(name="xpool", bufs=1) as xp,
            tc.tile_pool(name="sil1p", bufs=1) as sil1p,
            tc.tile_pool(name="sil2p", bufs=3) as sil2p,
            tc.tile_pool(name="outp", bufs=4) as outp,
            tc.tile_pool(name="psum", bufs=1, space="PSUM") as psump,
        ):
            w_t = wp.tile([P, NJ, KT, D], BF16)
            xu_t = xp.tile([P, KT, U], BF16)
            bias_t = constp.tile([P, NJ * KT], F32)
            sil1_t = sil1p.tile([P, KT, S], BF16)

            # --- input DMA: two parallel HWDGE issue streams, ordered by
            # first-use so the wires never carry "future" data during the
            # ramp. scalar ring: weights, split into m-group halves (phase 1
            # only consumes cols 0:512 of each W0 k-tile).
            for h in range(2):
                for k in range(KT):
                    nc.scalar.dma_start(
                        w_t[:, 0, k, h * CH : (h + 1) * CH],
                        W_d[0, k * P : (k + 1) * P, h * CH : (h + 1) * CH],
                    )
            nc.scalar.dma_start(
                w_t[:, 1, :, :], W_d[1].rearrange("(kt p) n -> p kt n", p=P)
            )
            nc.scalar.dma_start(
                w_t[:, 2, :, :], W_d[2].rearrange("(kt p) n -> p kt n", p=P)
            )
            # sync ring: activations (per-k chunk 0 paces the opening phase,
            # the rest lands long before it is needed)
            for k in range(KT):
                nc.sync.dma_start(xu_t[:, k, 0:CH], xu_d[:, k, 0:CH])
            nc.sync.dma_start(bias_t[:], b_d[:])
            nc.sync.dma_start(xu_t[:, :, CH:U], xu_d[:, :, CH:U])

            # --- shared experts, k-outer: phase = (j, col-chunk, m-group) ---
            # 8 PSUM banks = 2 sets x 4 m-tiles; sets alternate per phase so a
            # bank's silu has a full phase (~7us) to drain before reuse.
            for j in range(2):
                for c0, c1 in ((0, CH), (CH, S)):
                    for mg in range(2):
                        ps = [
                            psump.tile(
                                [P, CH], F32, tag=f"ps{mg * 4 + i}",
                                name=f"pss_{j}_{c0}_{mg}_{i}",
                            )
                            for i in range(4)
                        ]
                        for k in range(KT):
                            for i in range(4):
                                m = mg * 4 + i
                                nc.tensor.matmul(
                                    ps[i][:],
                                    w_t[:, j, k, m * P : (m + 1) * P],
                                    xu_t[:, k, c0:c1],
                                    start=(k == 0),
                                    stop=(k == KT - 1),
                                )
                        for i in range(4):
                            m = mg * 4 + i
                            bidx = j * KT + m
                            if j == 0:
                                nc.scalar.activation(
                                    sil1_t[:, m, c0:c1],
                                    ps[i][:],
                                    mybir.ActivationFunctionType.Silu,
                                    bias=bias_t[:, bidx : bidx + 1],
                                )
                            else:
                                sil2 = sil2p.tile(
                                    [P, CH], BF16, tag="sil2",
                                    name=f"sil2_{c0}_{m}",
                                )
                                nc.scalar.activation(
                                    sil2[:],
                                    ps[i][:],
                                    mybir.ActivationFunctionType.Silu,
                                    bias=bias_t[:, bidx : bidx + 1],
                                )
                                outs_t = outp.tile(
                                    [P, CH], BF16, tag="outs",
                                    name=f"outs_{c0}_{m}",
                                )
                                nc.vector.tensor_add(
                                    outs_t[:], sil1_t[:, m, c0:c1], sil2[:]
                                )
                                nc.sync.dma_start(
                                    outs_d[m * P : (m + 1) * P, c0:c1], outs_t[:]
                                )

            # --- routed expert, m-outer k-inner (mid-kernel, DMA caught up):
            # per (m,k) the 3 column chunks share one weight load; the final
            # chunk is the small extras block, so the tail drain is tiny.
            r_chunks = _chunk_ranges(0, U)
            tag_ctr = 0
            for m in range(KT):
                psums = []
                for ci, (c0, c1) in enumerate(r_chunks):
                    psums.append(
                        psump.tile(
                            [P, c1 - c0], F32, tag=f"ps{tag_ctr % 8}",
                            name=f"psr_{m}_{ci}",
                        )
                    )
                    tag_ctr += 1
                for k in range(KT):
                    lhsT = w_t[:, 2, k, m * P : (m + 1) * P]
                    for ci, (c0, c1) in enumerate(r_chunks):
                        nc.tensor.matmul(
                            psums[ci][:],
                            lhsT,
                            xu_t[:, k, c0:c1],
                            start=(k == 0),
                            stop=(k == KT - 1),
                        )
                bidx = 2 * KT + m
                for ci, (c0, c1) in enumerate(r_chunks):
                    outr_t = outp.tile(
                        [P, c1 - c0], BF16, tag="outr", name=f"outr_{m}_{ci}"
                    )
                    nc.scalar.activation(
                        outr_t[:],
                        psums[ci][:],
                        mybir.ActivationFunctionType.Silu,
                        bias=bias_t[:, bidx : bidx + 1],
                    )
                    nc.sync.dma_start(
                        outr_d[m * P : (m + 1) * P, c0:c1], outr_t[:]
                    )

    nc.compile()
    return nc


def _get_program(U):
    if U not in _program_cache:
        _program_cache[U] = _build_program(U)
    return _program_cache[U]


def kernel(x, expert_idx, Wr, br, Ws, bs):
    global last_results, last_nc, last_in_maps

    x = np.asarray(x, dtype=np.float32)
    idx = np.asarray(expert_idx).astype(np.int64)
    Wr = np.asarray(Wr, dtype=np.float32)
    br = np.asarray(br, dtype=np.float32)
    Ws = np.asarray(Ws, dtype=np.float32)
    bs = np.asarray(bs, dtype=np.float32)

    n_tokens = x.shape[0]
    assert x.shape == (N_CORES * S, D), f"unexpected x shape {x.shape}"

    # --- host-side "all-to-all": group tokens by expert ---
    order = np.argsort(idx, kind="stable")
    counts = np.bincount(idx, minlength=N_EXPERTS)
    offsets = np.zeros(N_EXPERTS + 1, dtype=np.int64)
    np.cumsum(counts, out=offsets[1:])

    x_sorted_bf = x[order].astype(ml_dtypes.bfloat16)

    # Core e's xu = [sorted window e*S:(e+1)*S | extras], where extras are
    # the routed (expert-e) tokens falling outside that window. The routed
    # phase computes all U columns with Wr[e]; only the expert-e ones are
    # kept, so the window+extras layout lets one x load serve both phases.
    extras = []
    cols = []  # per core: xu column of each routed token (sorted order)
    for e in range(N_CORES):
        p = np.arange(offsets[e], offsets[e + 1])
        inside = (p >= e * S) & (p < (e + 1) * S)
        ex = p[~inside]
        col = np.where(inside, p - e * S, 0)
        col[~inside] = S + np.arange(len(ex))
        extras.append(ex)
        cols.append(col)
    E = max(32, int(math.ceil(max(len(ex) for ex in extras) / 32)) * 32)
    U = S + E

    Wr_bf = Wr.astype(ml_dtypes.bfloat16)
    Ws_bf = Ws.astype(ml_dtypes.bfloat16)

    in_maps = []
    for e in range(N_CORES):
        # device layout [P, KT, U]: xu[p, k, u] = x_col[k*P + p, u], so every
        # DMA slice is a contiguous >=1KB run per partition
        xu = np.zeros((P, KT, U), dtype=ml_dtypes.bfloat16)
        xu[:, :, :S] = (
            x_sorted_bf[e * S : (e + 1) * S].T.reshape(KT, P, S).transpose(1, 0, 2)
        )
        if len(extras[e]):
            xu[:, :, S : S + len(extras[e])] = (
                x_sorted_bf[extras[e]].T
                .reshape(KT, P, len(extras[e]))
                .transpose(1, 0, 2)
            )

        W = np.empty((NJ, D, D), dtype=ml_dtypes.bfloat16)
        W[0] = Ws_bf[0]
        W[1] = Ws_bf[1]
        W[2] = Wr_bf[e]

        # b[p, j*KT + m] = bias_j[m*P + p]
        b = np.empty((P, NJ * KT), dtype=np.float32)
        for j, bias in enumerate((bs[0], bs[1], br[e])):
            b[:, j * KT : (j + 1) * KT] = bias.reshape(KT, P).T

        in_maps.append({"xu": xu, "W": W, "b": b})

    nc = _get_program(U)
    res = bass_utils.run_bass_kernel_spmd(nc, in_maps, core_ids=list(range(N_CORES)))
    last_results = res
    last_nc = nc
    last_in_maps = in_maps

    # combine in sorted-token space, then permute back to input order
    out_sorted = np.concatenate(
        [np.asarray(res.results[e]["outs"]).astype(np.float32).T for e in range(N_CORES)],
        axis=0,
    )
    for e in range(N_CORES):
        if counts[e] == 0:
            continue
        out_sorted[offsets[e] : offsets[e + 1]] += (
            np.asarray(res.results[e]["outr"]).astype(np.float32)[:, cols[e]].T
        )
    out = np.empty_like(out_sorted)
    out[order] = out_sorted
    return out[:n_tokens]
